# revision 36
# baseline (speedup 1.0000x reference)
"""Multi-head attention (B=4, N=2048, D=768, H=12, Dh=64) on 8 TRN2 NeuronCores.

Sharding: core c -> batch b = c//2, query rows half = c%2 (1024 rows each).
Each core computes all 12 heads for its (batch, query-half) against the full
2048-key sequence, so outputs are disjoint and no collective is needed.
The per-core input xT is the batch's x transposed to [768, 2048] with the
core's query half permuted to the front (attention is permutation-invariant
over keys, so K/V over the permuted sequence give identical results).

Per-core graph (bf16 matmuls, f32 accumulation):
  1. V projection first: V [2048, 12*65] in normal layout with a ones column
     per head (col 65h+64).
  2. Per head pair ht (heads 2ht, 2ht+1): project Q^T/K^T panels, then per
     head h:
       S^T[k, q] = K_h^T.T @ Q_h^T
       P^T = exp(0.125 * S^T)            (no max subtraction needed: scores
                                          are ~N(0,1), exp cannot overflow)
       U[65, q] = sum_k (V_h | 1).T @ P^T[k]   -- row 64 = softmax denom
       U[0:64] -> AOTU (unnormalized), U[64] -> D2 (denominators)
     then normalize the pair: R2 = 1/D2 (fast approx), broadcast to head
     partition rows via a K=2 matmul with the 0/1 selector E, and multiply
     into AOT.
  3. Final projection out[q, d] = AOT.T @ w_out + b_out, DMA out.
"""

import numpy as np

import concourse.bass as bass
import concourse.bacc as bacc
import concourse.mybir as mybir
import concourse.tile as tile
from concourse.bass_utils import run_bass_kernel_spmd

N_CORES = 8
B, N, D = 4, 2048, 768
H, DH = 12, 64
NQ = 1024           # query rows per core
COLS = 3 * D        # 2304 qkv columns
DT = D // 128       # 6 partition tiles of the model dim
NT = N // 128       # 16 key tiles
QT_TILES = NQ // 128  # 8 query tiles
VG = DH + 1         # 65: head group width in V (64 cols + ones)

F32 = mybir.dt.float32
BF16 = mybir.dt.bfloat16

PT_BUFS = 1


def build():
    nc = bacc.Bacc("TRN2", target_bir_lowering=False, debug=False,
                   num_devices=N_CORES)

    xT_d = nc.dram_tensor("xT", [D, N], BF16, kind="ExternalInput")
    wqkv_d = nc.dram_tensor("wqkv", [D, COLS], BF16, kind="ExternalInput")
    wout_d = nc.dram_tensor("wout", [D, D], BF16, kind="ExternalInput")
    bias_d = nc.dram_tensor("bias", [128, D], F32, kind="ExternalInput")
    out_d = nc.dram_tensor("out", [NQ, D], F32, kind="ExternalOutput")

    with tile.TileContext(nc) as tc:
        with tc.tile_pool(name="persist", bufs=1) as pp, \
             tc.tile_pool(name="small", bufs=1) as smallp, \
             tc.tile_pool(name="outs", bufs=6) as outsp:

            # ---- persistent tiles ----
            # 63 tail cols pad V so the attn@V stationary operand can be
            # a full 128-col slice (head h uses cols 65h..65h+128; output
            # rows 65..127 are junk and never read)
            V = [pp.tile([128, H * VG + 63], BF16, name=f"V{i}",
                         tag=f"V{i}") for i in range(NT)]
            AOT = [pp.tile([128, NQ], BF16, name=f"AOT{i}", tag=f"AOT{i}")
                   for i in range(DT)]
            AOTU = [pp.tile([128, NQ], BF16, name=f"AOTU{i}", tag=f"AOTU{i}")
                    for i in range(DT)]
            WO = [pp.tile([128, D], BF16, name=f"WO{i}", tag=f"WO{i}")
                  for i in range(DT)]
            BIAS = pp.tile([128, D], F32, name="BIAS", tag="BIAS")
            # E1/E2: selectors for broadcasting a head reciprocal row to its
            # 64 partition rows of the pair tile (K=1 matmuls, accumulated)
            E1 = pp.tile([1, 128], BF16, name="E1", tag="E1")
            E2 = pp.tile([1, 128], BF16, name="E2", tag="E2")

            nc.sync.dma_start(BIAS[:], bias_d.ap())
            nc.gpsimd.memset(E1[:], 0.0)
            nc.gpsimd.memset(E2[:], 0.0)
            nc.gpsimd.memset(E1[0:1, 0:DH], 1.0)
            nc.gpsimd.memset(E2[0:1, DH:128], 1.0)
            for i in range(DT):
                nc.sync.dma_start(WO[i][:], wout_d.ap()[i * 128:(i + 1) * 128, :])

            with tc.tile_pool(name="projin", bufs=1) as projin, \
                 tc.tile_pool(name="qk", bufs=1) as qkp, \
                 tc.tile_pool(name="pt", bufs=1) as ptp, \
                 tc.tile_pool(name="psA", bufs=2, space="PSUM") as psA, \
                 tc.tile_pool(name="psS", bufs=2, space="PSUM") as psS, \
                 tc.tile_pool(name="psO", bufs=2, space="PSUM") as psO:

                xT = [projin.tile([128, N], BF16, name=f"xT{i}", tag=f"xT{i}")
                      for i in range(DT)]
                for i in range(DT):
                    nc.sync.dma_start(xT[i][:],
                                      xT_d.ap()[i * 128:(i + 1) * 128, :])

                # ---- V [2048, 12*65]: normal layout, x^T stationary ----
                for vp in range(3):  # panels of 256 v-cols = 4 heads
                    co = 2 * D + vp * 256
                    wv = [projin.tile([128, 256], BF16, name=f"wv{d}",
                                      tag=f"wv{d}", bufs=2)
                          for d in range(DT)]
                    for d in range(DT):
                        nc.sync.dma_start(
                            wv[d][:],
                            wqkv_d.ap()[d * 128:(d + 1) * 128, co:co + 256])
                    for t in range(NT):
                        ps = psA.tile([128, 512], F32, name="psA", tag="psA")
                        for d in range(DT):
                            nc.tensor.matmul(
                                ps[:, :256],
                                xT[d][:, t * 128:(t + 1) * 128],
                                wv[d][:],
                                start=(d == 0), stop=(d == DT - 1))
                        dst = V[t][:, 0:H * VG].rearrange(
                            "p (h c) -> p h c", c=VG)
                        nc.vector.tensor_copy(
                            dst[:, vp * 4:(vp + 1) * 4, 0:DH],
                            ps[:, :256].rearrange("p (h c) -> p h c", c=DH))
                for t in range(NT):
                    ones = V[t][:, 0:H * VG].rearrange(
                        "p (h c) -> p h c", c=VG)[:, :, DH:VG]
                    nc.gpsimd.memset(ones, 1.0)
                    nc.gpsimd.memset(V[t][:, H * VG:], 0.0)

                # ---- per head pair: project Q^T/K^T, attention ----
                def normalize_pair(ht, DD, rpool=None, rtag="psA"):
                    # recip of denominators, broadcast to the pair's 128
                    # partition rows via accumulated K=1 matmuls, multiply
                    RB = []
                    for j in range(2):
                        rf = smallp.tile([1, NQ], F32, name=f"Rf{j}",
                                         tag=f"Rf{j}", bufs=2)
                        rb = smallp.tile([1, NQ], BF16, name=f"Rb{j}",
                                         tag=f"Rb{j}", bufs=2)
                        nc.vector.reciprocal_approx_fast(rf[:], DD[j][:])
                        nc.vector.tensor_copy(rb[:], rf[:])
                        RB.append(rb)
                    for qb in range(2):
                        qs = slice(qb * 512, (qb + 1) * 512)
                        rbp = (rpool or psA).tile(
                            [128, 512], F32, name="rbp", tag=rtag,
                            bufs=(1 if rpool is not None else None))
                        nc.tensor.matmul(rbp[:], E1[:], RB[0][:, qs],
                                         start=True, stop=False)
                        nc.tensor.matmul(rbp[:], E2[:], RB[1][:, qs],
                                         start=False, stop=True)
                        nc.vector.tensor_mul(
                            AOT[ht][:, qs], AOTU[ht][:, qs], rbp[:])

                # attn@V for head h as a generator: each step emits the two
                # qb matmuls for one k tile, so steps can be interleaved
                # into the next head's scores loop (the PE engine queue is
                # in-order; emission order is execution order)
                def attnv_gen(h, PT, DD):
                    ht, hp = divmod(h, 2)
                    po = [psO.tile([128, 512], F32, name=f"psO{qb}",
                                   tag="psO") for qb in range(2)]
                    for k in range(NT):
                        for qb in range(2):
                            nc.tensor.matmul(
                                po[qb][:],
                                V[k][:, h * VG:h * VG + 128],
                                PT[k][:, qb * 512:(qb + 1) * 512],
                                start=(k == 0), stop=(k == NT - 1))
                        yield
                    for qb in range(2):
                        qs = slice(qb * 512, (qb + 1) * 512)
                        nc.vector.tensor_copy(
                            AOTU[ht][hp * DH:(hp + 1) * DH, qs],
                            po[qb][0:DH, :])
                        nc.vector.tensor_copy(DD[hp][0:1, qs],
                                              po[qb][VG - 1:VG, :])

                filler = [None]

                def fill(n=1):
                    if filler[0] is None:
                        return
                    for _ in range(n):
                        try:
                            next(filler[0])
                        except StopIteration:
                            filler[0] = None
                            return

                def drain():
                    while filler[0] is not None:
                        fill()

                pending = []  # [(ht, DD)] awaiting normalize (2 deep: a
                # pair's DD copies are only emitted once its last head's
                # attn@V drains, during the NEXT pair's scores loop)
                for ht in range(DT):
                    # per-head Q with the other head's 64 rows zeroed, so
                    # scores run as uniform full-array [128,128,512] matmuls
                    # against the shared K^T pair tile (the other head's K
                    # rows meet zeroed Q rows and contribute nothing)
                    QZ = [qkp.tile([128, NQ], BF16, name=f"QZ{j}",
                                   tag=f"QZ{j}", bufs=2) for j in range(2)]
                    KTp = qkp.tile([128, N], BF16, name="KTp", tag="KTp",
                                   bufs=2)
                    nc.gpsimd.memset(QZ[0][DH:128, :], 0.0)
                    nc.gpsimd.memset(QZ[1][0:DH, :], 0.0)
                    for (dst, co, nn) in ((None, ht * 128, NQ),
                                          (KTp, D + ht * 128, N)):
                        wp = [projin.tile([128, 128], BF16, name=f"wqk{d}",
                                          tag=f"wqk{d}", bufs=2)
                              for d in range(DT)]
                        for d in range(DT):
                            nc.sync.dma_start(
                                wp[d][:],
                                wqkv_d.ap()[d * 128:(d + 1) * 128,
                                            co:co + 128])
                        for nb in range(nn // 512):
                            fill()
                            ps = psA.tile([128, 512], F32, name="psA",
                                          tag="psA")
                            for d in range(DT):
                                nc.tensor.matmul(
                                    ps[:],
                                    wp[d][:],
                                    xT[d][:, nb * 512:(nb + 1) * 512],
                                    start=(d == 0), stop=(d == DT - 1))
                            nbs = slice(nb * 512, (nb + 1) * 512)
                            if dst is not None:
                                nc.vector.tensor_copy(dst[:, nbs], ps[:])
                            else:
                                nc.vector.tensor_copy(
                                    QZ[0][0:DH, nbs], ps[0:DH, :])
                                nc.vector.tensor_copy(
                                    QZ[1][DH:128, nbs], ps[DH:128, :])

                    # normalize lands two pairs later so its DVE chain and
                    # the attn@V drain overlap projection instead of
                    # stalling the in-order PE queue
                    while len(pending) > 1:
                        normalize_pair(*pending.pop(0))

                    DD = [smallp.tile([1, NQ], F32, name=f"DD{j}",
                                      tag=f"DD{j}", bufs=3)
                          for j in range(2)]
                    for hp in range(2):
                        h = 2 * ht + hp
                        PT = [ptp.tile([128, NQ], BF16, name=f"PT{k}",
                                       tag=f"PT{k}", bufs=PT_BUFS)
                              for k in range(NT)]
                        for k in range(NT):
                            # one attn@V k-step of the previous head rides
                            # in the ACT-gated gap of each scores step; it
                            # also frees PT[k] right before exp rewrites it
                            fill()
                            ps = psS.tile([128, NQ], F32, name="psS",
                                          tag="psS")
                            for qb in range(2):
                                nc.tensor.matmul(
                                    ps[:, qb * 512:(qb + 1) * 512],
                                    KTp[:, k * 128:(k + 1) * 128],
                                    QZ[hp][:, qb * 512:(qb + 1) * 512],
                                    start=True, stop=True)
                            nc.scalar.activation(
                                PT[k][:], ps[:],
                                mybir.ActivationFunctionType.Exp, scale=0.125)
                        drain()
                        # on the last pair there is no later projection to
                        # absorb pending normalizes; emit them as soon as
                        # their attn@V has drained instead of at the end
                        if ht == DT - 1 and hp == 0 and pending:
                            normalize_pair(*pending.pop(0))
                        filler[0] = attnv_gen(h, PT, DD)
                    pending.append((ht, DD))
                drain()
                last_pending = pending

                # ---- final projection (inside the attention scope's
                # closures; own PSUM pool opens after attention pools) ----
                with tc.tile_pool(name="psC", bufs=7, space="PSUM") as psC:
                    groups = [(qt, fo, fsz) for qt in range(QT_TILES)
                              for (fo, fsz) in ((0, 512), (512, 256))]
                    # hoist the AOT[5]-free partials of the first 4 groups
                    # so they run during the last normalize's DVE chain
                    hoisted = {}
                    for (qt, fo, fsz) in groups[:7]:
                        ps = psC.tile([128, 512], F32, name="psF", tag="psF")
                        hoisted[(qt, fo, fsz)] = ps
                        for i in range(DT - 1):
                            nc.tensor.matmul(
                                ps[:, :fsz],
                                AOT[i][:, qt * 128:(qt + 1) * 128],
                                WO[i][:, fo:fo + fsz],
                                start=(i == 0), stop=False)
                    for p in last_pending:
                        normalize_pair(*p, rpool=psC, rtag="rbp")
                    for (qt, fo, fsz) in groups:
                        ps = hoisted.get((qt, fo, fsz))
                        i0 = DT - 1 if ps is not None else 0
                        if ps is None:
                            ps = psC.tile([128, 512], F32, name="psF",
                                          tag="psF")
                        for i in range(i0, DT):
                            nc.tensor.matmul(
                                ps[:, :fsz],
                                AOT[i][:, qt * 128:(qt + 1) * 128],
                                WO[i][:, fo:fo + fsz],
                                start=(i == 0), stop=(i == DT - 1))
                        ot = outsp.tile([128, 512], F32, name="ot", tag="ot")
                        nc.vector.tensor_add(
                            ot[:, :fsz], ps[:, :fsz], BIAS[:, fo:fo + fsz])
                        nc.sync.dma_start(
                            out_d.ap()[qt * 128:(qt + 1) * 128, fo:fo + fsz],
                            ot[:, :fsz])

    nc.compile()
    return nc


_NC = None


def _get_nc():
    global _NC
    if _NC is None:
        _NC = build()
    return _NC


def make_in_maps(x, w_qkv, w_out, b_out):
    import ml_dtypes
    x = np.asarray(x, np.float32)
    w_qkv = np.ascontiguousarray(np.asarray(w_qkv, ml_dtypes.bfloat16))
    w_out = np.ascontiguousarray(np.asarray(w_out, ml_dtypes.bfloat16))
    bias = np.ascontiguousarray(
        np.broadcast_to(np.asarray(b_out, np.float32)[None, :], (128, D)))
    in_maps = []
    for c in range(N_CORES):
        b, half = divmod(c, 2)
        xb = x[b]
        qoff = half * NQ
        # query half first; key order permutation is harmless
        xperm = np.vstack([xb[qoff:qoff + NQ], xb[NQ - qoff:2 * NQ - qoff]])
        in_maps.append({
            "xT": np.ascontiguousarray(xperm.T.astype(ml_dtypes.bfloat16)),
            "wqkv": w_qkv,
            "wout": w_out,
            "bias": bias,
        })
    return in_maps


def run(in_maps, trace=False, **kw):
    return run_bass_kernel_spmd(_get_nc(), in_maps,
                                core_ids=list(range(N_CORES)),
                                trace=trace, **kw)


def assemble(results):
    out = np.empty((B, N, D), np.float32)
    for c in range(N_CORES):
        b, half = divmod(c, 2)
        out[b, half * NQ:(half + 1) * NQ, :] = results[c]["out"]
    return out


def kernel(x, w_qkv, w_out, b_out):
    res = run(make_in_maps(x, w_qkv, w_out, b_out))
    return assemble(res.results)


# revision 37
# speedup vs baseline: 1.0052x; 1.0052x over previous
"""Multi-head attention (B=4, N=2048, D=768, H=12, Dh=64) on 8 TRN2 NeuronCores.

Sharding: core c -> batch b = c//2, query rows half = c%2 (1024 rows each).
Each core computes all 12 heads for its (batch, query-half) against the full
2048-key sequence, so outputs are disjoint and no collective is needed.
The per-core input xT is the batch's x transposed to [768, 2048] with the
core's query half permuted to the front (attention is permutation-invariant
over keys, so K/V over the permuted sequence give identical results).

Per-core graph (bf16 matmuls, f32 accumulation):
  1. V projection first: V [2048, 12*65] in normal layout with a ones column
     per head (col 65h+64).
  2. Per head pair ht (heads 2ht, 2ht+1): project Q^T/K^T panels, then per
     head h:
       S^T[k, q] = K_h^T.T @ Q_h^T
       P^T = exp(0.125 * S^T)            (no max subtraction needed: scores
                                          are ~N(0,1), exp cannot overflow)
       U[65, q] = sum_k (V_h | 1).T @ P^T[k]   -- row 64 = softmax denom
       U[0:64] -> AOTU (unnormalized), U[64] -> D2 (denominators)
     then normalize the pair: R2 = 1/D2 (fast approx), broadcast to head
     partition rows via a K=2 matmul with the 0/1 selector E, and multiply
     into AOT.
  3. Final projection out[q, d] = AOT.T @ w_out + b_out, DMA out.
"""

import numpy as np

import concourse.bass as bass
import concourse.bacc as bacc
import concourse.mybir as mybir
import concourse.tile as tile
from concourse.bass_utils import run_bass_kernel_spmd

N_CORES = 8
B, N, D = 4, 2048, 768
H, DH = 12, 64
NQ = 1024           # query rows per core
COLS = 3 * D        # 2304 qkv columns
DT = D // 128       # 6 partition tiles of the model dim
NT = N // 128       # 16 key tiles
QT_TILES = NQ // 128  # 8 query tiles
VG = DH + 1         # 65: head group width in V (64 cols + ones)

F32 = mybir.dt.float32
BF16 = mybir.dt.bfloat16

PT_BUFS = 1


def build():
    nc = bacc.Bacc("TRN2", target_bir_lowering=False, debug=False,
                   num_devices=N_CORES)

    xT_d = nc.dram_tensor("xT", [D, N], BF16, kind="ExternalInput")
    wqkv_d = nc.dram_tensor("wqkv", [D, COLS], BF16, kind="ExternalInput")
    wout_d = nc.dram_tensor("wout", [D, D], BF16, kind="ExternalInput")
    bias_d = nc.dram_tensor("bias", [128, D], F32, kind="ExternalInput")
    out_d = nc.dram_tensor("out", [NQ, D], F32, kind="ExternalOutput")

    with tile.TileContext(nc) as tc:
        with tc.tile_pool(name="persist", bufs=1) as pp, \
             tc.tile_pool(name="small", bufs=1) as smallp, \
             tc.tile_pool(name="outs", bufs=6) as outsp:

            # ---- persistent tiles ----
            # 63 tail cols pad V so the attn@V stationary operand can be
            # a full 128-col slice (head h uses cols 65h..65h+128; output
            # rows 65..127 are junk and never read)
            V = [pp.tile([128, H * VG + 63], BF16, name=f"V{i}",
                         tag=f"V{i}") for i in range(NT)]
            AOT = [pp.tile([128, NQ], BF16, name=f"AOT{i}", tag=f"AOT{i}")
                   for i in range(DT)]
            AOTU = [pp.tile([128, NQ], BF16, name=f"AOTU{i}", tag=f"AOTU{i}")
                    for i in range(DT)]
            WO = [pp.tile([128, D], BF16, name=f"WO{i}", tag=f"WO{i}")
                  for i in range(DT)]
            BIAS = pp.tile([128, D], F32, name="BIAS", tag="BIAS")
            # E1/E2: selectors for broadcasting a head reciprocal row to its
            # 64 partition rows of the pair tile (K=1 matmuls, accumulated)
            E1 = pp.tile([1, 128], BF16, name="E1", tag="E1")
            E2 = pp.tile([1, 128], BF16, name="E2", tag="E2")

            nc.sync.dma_start(BIAS[:], bias_d.ap())
            nc.gpsimd.memset(E1[:], 0.0)
            nc.gpsimd.memset(E2[:], 0.0)
            nc.gpsimd.memset(E1[0:1, 0:DH], 1.0)
            nc.gpsimd.memset(E2[0:1, DH:128], 1.0)
            for i in range(DT):
                nc.sync.dma_start(WO[i][:], wout_d.ap()[i * 128:(i + 1) * 128, :])

            with tc.tile_pool(name="projin", bufs=1) as projin, \
                 tc.tile_pool(name="qk", bufs=1) as qkp, \
                 tc.tile_pool(name="pt", bufs=1) as ptp, \
                 tc.tile_pool(name="psA", bufs=2, space="PSUM") as psA, \
                 tc.tile_pool(name="psS", bufs=2, space="PSUM") as psS, \
                 tc.tile_pool(name="psO", bufs=2, space="PSUM") as psO:

                xT = [projin.tile([128, N], BF16, name=f"xT{i}", tag=f"xT{i}")
                      for i in range(DT)]
                for i in range(DT):
                    nc.sync.dma_start(xT[i][:],
                                      xT_d.ap()[i * 128:(i + 1) * 128, :])

                # ---- V [2048, 12*65]: normal layout, x^T stationary ----
                for vp in range(3):  # panels of 256 v-cols = 4 heads
                    co = 2 * D + vp * 256
                    wv = [projin.tile([128, 256], BF16, name=f"wv{d}",
                                      tag=f"wv{d}", bufs=2)
                          for d in range(DT)]
                    for d in range(DT):
                        nc.sync.dma_start(
                            wv[d][:],
                            wqkv_d.ap()[d * 128:(d + 1) * 128, co:co + 256])
                    for t in range(NT):
                        ps = psA.tile([128, 512], F32, name="psA", tag="psA")
                        for d in range(DT):
                            nc.tensor.matmul(
                                ps[:, :256],
                                xT[d][:, t * 128:(t + 1) * 128],
                                wv[d][:],
                                start=(d == 0), stop=(d == DT - 1))
                        dst = V[t][:, 0:H * VG].rearrange(
                            "p (h c) -> p h c", c=VG)
                        nc.vector.tensor_copy(
                            dst[:, vp * 4:(vp + 1) * 4, 0:DH],
                            ps[:, :256].rearrange("p (h c) -> p h c", c=DH))
                for t in range(NT):
                    ones = V[t][:, 0:H * VG].rearrange(
                        "p (h c) -> p h c", c=VG)[:, :, DH:VG]
                    nc.gpsimd.memset(ones, 1.0)
                    nc.gpsimd.memset(V[t][:, H * VG:], 0.0)

                # ---- per head pair: project Q^T/K^T, attention ----
                def normalize_pair(ht, DD, rpool=None, rtag="psA"):
                    # recip of denominators, broadcast to the pair's 128
                    # partition rows via accumulated K=1 matmuls, multiply
                    RB = []
                    for j in range(2):
                        rf = smallp.tile([1, NQ], F32, name=f"Rf{j}",
                                         tag=f"Rf{j}", bufs=2)
                        rb = smallp.tile([1, NQ], BF16, name=f"Rb{j}",
                                         tag=f"Rb{j}", bufs=2)
                        nc.vector.reciprocal_approx_fast(rf[:], DD[j][:])
                        nc.vector.tensor_copy(rb[:], rf[:])
                        RB.append(rb)
                    for qb in range(2):
                        qs = slice(qb * 512, (qb + 1) * 512)
                        rbp = (rpool or psA).tile([128, 512], F32,
                                                  name="rbp", tag=rtag)
                        nc.tensor.matmul(rbp[:], E1[:], RB[0][:, qs],
                                         start=True, stop=False)
                        nc.tensor.matmul(rbp[:], E2[:], RB[1][:, qs],
                                         start=False, stop=True)
                        nc.vector.tensor_mul(
                            AOT[ht][:, qs], AOTU[ht][:, qs], rbp[:])

                # attn@V for head h as a generator: each step emits the two
                # qb matmuls for one k tile, so steps can be interleaved
                # into the next head's scores loop (the PE engine queue is
                # in-order; emission order is execution order)
                def attnv_gen(h, PT, DD):
                    ht, hp = divmod(h, 2)
                    po = [psO.tile([128, 512], F32, name=f"psO{qb}",
                                   tag="psO") for qb in range(2)]
                    for k in range(NT):
                        for qb in range(2):
                            nc.tensor.matmul(
                                po[qb][:],
                                V[k][:, h * VG:h * VG + 128],
                                PT[k][:, qb * 512:(qb + 1) * 512],
                                start=(k == 0), stop=(k == NT - 1))
                        yield
                    for qb in range(2):
                        qs = slice(qb * 512, (qb + 1) * 512)
                        nc.vector.tensor_copy(
                            AOTU[ht][hp * DH:(hp + 1) * DH, qs],
                            po[qb][0:DH, :])
                        nc.vector.tensor_copy(DD[hp][0:1, qs],
                                              po[qb][VG - 1:VG, :])

                filler = [None]

                def fill(n=1):
                    if filler[0] is None:
                        return
                    for _ in range(n):
                        try:
                            next(filler[0])
                        except StopIteration:
                            filler[0] = None
                            return

                def drain():
                    while filler[0] is not None:
                        fill()

                pending = []  # [(ht, DD)] awaiting normalize (2 deep: a
                # pair's DD copies are only emitted once its last head's
                # attn@V drains, during the NEXT pair's scores loop)
                for ht in range(DT):
                    # per-head Q with the other head's 64 rows zeroed, so
                    # scores run as uniform full-array [128,128,512] matmuls
                    # against the shared K^T pair tile (the other head's K
                    # rows meet zeroed Q rows and contribute nothing)
                    QZ = [qkp.tile([128, NQ], BF16, name=f"QZ{j}",
                                   tag=f"QZ{j}", bufs=2) for j in range(2)]
                    KTp = qkp.tile([128, N], BF16, name="KTp", tag="KTp",
                                   bufs=2)
                    nc.gpsimd.memset(QZ[0][DH:128, :], 0.0)
                    nc.gpsimd.memset(QZ[1][0:DH, :], 0.0)
                    for (dst, co, nn) in ((None, ht * 128, NQ),
                                          (KTp, D + ht * 128, N)):
                        wp = [projin.tile([128, 128], BF16, name=f"wqk{d}",
                                          tag=f"wqk{d}", bufs=2)
                              for d in range(DT)]
                        for d in range(DT):
                            nc.sync.dma_start(
                                wp[d][:],
                                wqkv_d.ap()[d * 128:(d + 1) * 128,
                                            co:co + 128])
                        for nb in range(nn // 512):
                            fill()
                            ps = psA.tile([128, 512], F32, name="psA",
                                          tag="psA")
                            for d in range(DT):
                                nc.tensor.matmul(
                                    ps[:],
                                    wp[d][:],
                                    xT[d][:, nb * 512:(nb + 1) * 512],
                                    start=(d == 0), stop=(d == DT - 1))
                            nbs = slice(nb * 512, (nb + 1) * 512)
                            if dst is not None:
                                nc.vector.tensor_copy(dst[:, nbs], ps[:])
                            else:
                                nc.vector.tensor_copy(
                                    QZ[0][0:DH, nbs], ps[0:DH, :])
                                nc.vector.tensor_copy(
                                    QZ[1][DH:128, nbs], ps[DH:128, :])

                    # normalize lands two pairs later so its DVE chain and
                    # the attn@V drain overlap projection instead of
                    # stalling the in-order PE queue
                    while len(pending) > 1:
                        normalize_pair(*pending.pop(0))

                    DD = [smallp.tile([1, NQ], F32, name=f"DD{j}",
                                      tag=f"DD{j}", bufs=3)
                          for j in range(2)]
                    for hp in range(2):
                        h = 2 * ht + hp
                        PT = [ptp.tile([128, NQ], BF16, name=f"PT{k}",
                                       tag=f"PT{k}", bufs=PT_BUFS)
                              for k in range(NT)]
                        for k in range(NT):
                            # one attn@V k-step of the previous head rides
                            # in the ACT-gated gap of each scores step; it
                            # also frees PT[k] right before exp rewrites it
                            fill()
                            ps = psS.tile([128, NQ], F32, name="psS",
                                          tag="psS")
                            for qb in range(2):
                                nc.tensor.matmul(
                                    ps[:, qb * 512:(qb + 1) * 512],
                                    KTp[:, k * 128:(k + 1) * 128],
                                    QZ[hp][:, qb * 512:(qb + 1) * 512],
                                    start=True, stop=True)
                            nc.scalar.activation(
                                PT[k][:], ps[:],
                                mybir.ActivationFunctionType.Exp, scale=0.125)
                        drain()
                        # on the last pair there is no later projection to
                        # absorb pending normalizes; emit them as soon as
                        # their attn@V has drained instead of at the end
                        if ht == DT - 1 and hp == 0 and pending:
                            normalize_pair(*pending.pop(0))
                        filler[0] = attnv_gen(h, PT, DD)
                    pending.append((ht, DD))
                drain()
                last_pending = pending

                # ---- final projection (inside the attention scope's
                # closures; own PSUM pool opens after attention pools) ----
                with tc.tile_pool(name="psC", bufs=4, space="PSUM") as psC:
                    groups = [(qt, fo, fsz) for qt in range(QT_TILES)
                              for (fo, fsz) in ((0, 512), (512, 256))]
                    # hoist the AOT[5]-free partials of the first 4 groups
                    # so they run during the last normalize's DVE chain
                    hoisted = {}
                    for (qt, fo, fsz) in groups[:4]:
                        ps = psC.tile([128, 512], F32, name="psF", tag="psF")
                        hoisted[(qt, fo, fsz)] = ps
                        for i in range(DT - 1):
                            nc.tensor.matmul(
                                ps[:, :fsz],
                                AOT[i][:, qt * 128:(qt + 1) * 128],
                                WO[i][:, fo:fo + fsz],
                                start=(i == 0), stop=False)
                    for p in last_pending:
                        normalize_pair(*p, rpool=psC, rtag="rbp")
                    for (qt, fo, fsz) in groups:
                        ps = hoisted.get((qt, fo, fsz))
                        i0 = DT - 1 if ps is not None else 0
                        if ps is None:
                            ps = psC.tile([128, 512], F32, name="psF",
                                          tag="psF")
                        for i in range(i0, DT):
                            nc.tensor.matmul(
                                ps[:, :fsz],
                                AOT[i][:, qt * 128:(qt + 1) * 128],
                                WO[i][:, fo:fo + fsz],
                                start=(i == 0), stop=(i == DT - 1))
                        ot = outsp.tile([128, 512], F32, name="ot", tag="ot")
                        nc.vector.tensor_add(
                            ot[:, :fsz], ps[:, :fsz], BIAS[:, fo:fo + fsz])
                        nc.sync.dma_start(
                            out_d.ap()[qt * 128:(qt + 1) * 128, fo:fo + fsz],
                            ot[:, :fsz])

    nc.compile()
    return nc


_NC = None


def _get_nc():
    global _NC
    if _NC is None:
        _NC = build()
    return _NC


def make_in_maps(x, w_qkv, w_out, b_out):
    import ml_dtypes
    x = np.asarray(x, np.float32)
    w_qkv = np.ascontiguousarray(np.asarray(w_qkv, ml_dtypes.bfloat16))
    w_out = np.ascontiguousarray(np.asarray(w_out, ml_dtypes.bfloat16))
    bias = np.ascontiguousarray(
        np.broadcast_to(np.asarray(b_out, np.float32)[None, :], (128, D)))
    in_maps = []
    for c in range(N_CORES):
        b, half = divmod(c, 2)
        xb = x[b]
        qoff = half * NQ
        # query half first; key order permutation is harmless
        xperm = np.vstack([xb[qoff:qoff + NQ], xb[NQ - qoff:2 * NQ - qoff]])
        in_maps.append({
            "xT": np.ascontiguousarray(xperm.T.astype(ml_dtypes.bfloat16)),
            "wqkv": w_qkv,
            "wout": w_out,
            "bias": bias,
        })
    return in_maps


def run(in_maps, trace=False, **kw):
    return run_bass_kernel_spmd(_get_nc(), in_maps,
                                core_ids=list(range(N_CORES)),
                                trace=trace, **kw)


def assemble(results):
    out = np.empty((B, N, D), np.float32)
    for c in range(N_CORES):
        b, half = divmod(c, 2)
        out[b, half * NQ:(half + 1) * NQ, :] = results[c]["out"]
    return out


def kernel(x, w_qkv, w_out, b_out):
    res = run(make_in_maps(x, w_qkv, w_out, b_out))
    return assemble(res.results)
